# revision 15
# baseline (speedup 1.0000x reference)
"""Trainium2 Bass kernel for ALayer: out = x * box3x3(sigmoid(conv3x3(relu(conv3x3(x,w1)),w2))).

Sharding: pure data parallel over batch (32 images -> 4 per core x 8 cores).

Per-core plan:
  - x loaded per 2-image pack as [128=(img2,c64), guarded spatial] bf16 (DMA cast),
    in 8 row-chunks; an fp8e4m3 copy x8 [128, 2, BUF_W] (both packs in the DoubleRow
    K-segment dim) is made by chunked ScalarE copy-casts (keeps DMA queues free).
  - conv1 interior: fp8 DoubleRow matmuls, K=256=(img4,c64), M=64=(pack2,img2,co16),
    9 taps accumulated in PSUM, tap-outer over groups of 4 tiles to amortize weight
    loads; full-width rows (contiguous rhs, wrap garbage at x=0/127) + bf16 per-pack
    edge-column matmuls with only the valid taps.
  - relu+cast eviction (DVE): ONE [64,512] op per tile into h64 [64=(pack,img,co), .]
    fp8; dy-replicas hrep4 [96=(dy3,img2,c16), 2seg, .] built by 6 chunked shifted
    DMAs; conv2 = 3 fp8-DR matmuls (dx), K=192, both packs at once.
  - sigmoid evictions (ACT) into group 0 of per-pack Arep [128-part, 18 live];
    8 shifted DMA replicas; box+channel-broadcast in ONE bf16 matmul: K=128
    (zero-padded), M=128=(img2,c64).
  - final multiply in-place on the x tile (DVE), stored with bf16->f32 DMA cast.

All stages emitted at 16-row chunk granularity so the Tile scheduler overlaps
DMA / PE / ACT / DVE across stage boundaries.
"""

import numpy as np
import ml_dtypes

import concourse.bacc as bacc
import concourse.mybir as mybir
from concourse.tile import TileContext
from concourse.bass_utils import run_bass_kernel_spmd

BF16 = mybir.dt.bfloat16
F32 = mybir.dt.float32
FP8 = mybir.dt.float8e4

# Problem constants (hardcoded; kernel.py must be self-contained)
B, C, H, W = 32, 64, 128, 128
N_CORES = 8
B_LOC = B // N_CORES            # 4 images per core
PACKS = B_LOC // 2              # 2 two-image packs
S = H * W                       # 16384
FG = 160                        # front zero-guard (>=129 needed)
BG = 288
BUF_W = FG + S + BG             # 16832 (16-aligned for DoubleRow AP steps)
Y4 = 4                          # output rows per psum tile
NT = H // Y4                    # 32 row tiles
GRP = 4                         # conv1 tiles per weight-reuse group
NCH = 8                         # row-chunks (16 rows each)
CROWS = H // NCH                # 16
TPC = NT // NCH                 # interior tiles per chunk (4)
XI = W - 2                      # 126 interior columns
TAPORD = (4, 0, 1, 2, 3, 5, 6, 7, 8)   # Arep group -> tap index (center first)


def _pos(y, x):
    return FG + y * W + x


def _host_weights(w1, w2):
    """Precompute SBUF weight layouts (block-diagonal over image packing)."""
    w1 = np.asarray(w1, np.float32)     # [16, 64, 3, 3]
    w2 = np.asarray(w2, np.float32)     # [1, 16, 3, 3]
    bf = ml_dtypes.bfloat16
    f8 = mybir.dt.np(FP8)

    # conv1 DoubleRow: lhsT[p=(il2,c64), seg2, t, m=(seg'2,il'2,co16)]
    #   = w1[co, c, ky, kx] * (seg'==seg) * (il'==il);  img = 2*seg + il
    w1D = np.zeros((128, 2, 9, 64), np.float32)
    for il in range(2):
        for seg in range(2):
            m0 = seg * 32 + il * 16
            for t in range(9):
                ky, kx = divmod(t, 3)
                w1D[il * 64:(il + 1) * 64, seg, t, m0:m0 + 16] = w1[:, :, ky, kx].T
    # conv1 edges (bf16, per pack): lhsT[p=(i2,c64), t, m=(il2,co16)]
    w1L = np.zeros((128, 9, 32), np.float32)
    for i in range(2):
        for t in range(9):
            ky, kx = divmod(t, 3)
            w1L[i * 64:(i + 1) * 64, t, i * 16:(i + 1) * 16] = w1[:, :, ky, kx].T
    # conv2 DoubleRow: lhsT[p=(g3,il2,c16), seg2, kx, m] = w2[0,c,g,kx] at
    # m = seg*32 + il (M=64 padded; psum rows {0,1,32,33} -> per-pack evictions)
    w2D = np.zeros((96, 2, 3, 64), np.float32)
    for g in range(3):
        for il in range(2):
            for seg in range(2):
                for kx in range(3):
                    w2D[g * 32 + il * 16:g * 32 + il * 16 + 16, seg, kx,
                        seg * 32 + il] = w2[0, :, g, kx]
    # conv2 edges (fp8 non-DR, per seg): lhsT[p=(g3,i2,c16), kx, il2]
    w2L = np.zeros((96, 3, 2), np.float32)
    for g in range(3):
        for i in range(2):
            for kx in range(3):
                w2L[g * 32 + i * 16:g * 32 + (i + 1) * 16, kx, i] = w2[0, :, g, kx]
    # box+bcast: lhsT[p=(group9,i2)+pad, e, m=(il2,c64)] = tap-allowed * (i == il)
    onesL = np.zeros((128, 3, 128), np.float32)
    for j, t in enumerate(TAPORD):
        kx = t % 3
        for i in range(2):
            p = 2 * j + i
            onesL[p, 0, i * 64:(i + 1) * 64] = 1.0
            if kx >= 1:
                onesL[p, 1, i * 64:(i + 1) * 64] = 1.0
            if kx <= 1:
                onesL[p, 2, i * 64:(i + 1) * 64] = 1.0
    return (w1D.astype(f8), w1L.astype(bf), w2D.astype(f8), w2L.astype(f8),
            onesL.astype(bf))


def _view(t, off, rows, cols):
    """AP over [partitions, (rows @ W-stride, cols @ 1)] at free offset `off`."""
    return t[:, off:off + (rows - 1) * W + 128].rearrange(
        "p (y x) -> p y x", x=W)[:, :, :cols]


def _view3(t, seg, off, rows, cols):
    """Same for a [P, 2, BUF_W] tensor at segment `seg`."""
    return t[:, seg, off:off + (rows - 1) * W + 128].rearrange(
        "p (y x) -> p y x", x=W)[:, :, :cols]


def _split(a, b, n=NCH):
    step = (b - a) // n
    cuts = [a + i * step for i in range(n)] + [b]
    return [(cuts[i], cuts[i + 1]) for i in range(n)]


def _build_nc():
    nc = bacc.Bacc(None, target_bir_lowering=False, debug=False)

    x_ext = nc.declare_dram_parameter("x", [B_LOC, C, H, W], F32, isOutput=False)
    out_ext = nc.declare_dram_parameter("out", [B_LOC, C, H, W], F32, isOutput=True)
    w1D_ext = nc.declare_dram_parameter("w1D", [128, 2, 9, 64], FP8, isOutput=False)
    w1L_ext = nc.declare_dram_parameter("w1L", [128, 9, 32], BF16, isOutput=False)
    w2D_ext = nc.declare_dram_parameter("w2D", [96, 2, 3, 64], FP8, isOutput=False)
    w2L_ext = nc.declare_dram_parameter("w2L", [96, 3, 2], FP8, isOutput=False)
    onesL_ext = nc.declare_dram_parameter("onesL", [128, 3, 128], BF16, isOutput=False)

    with TileContext(nc) as tc:
        with (
            tc.tile_pool(name="wpool", bufs=1) as wpool,
            tc.tile_pool(name="xpool", bufs=2) as xpool,
            tc.tile_pool(name="hpool", bufs=1) as hpool,
            tc.tile_pool(name="bigpool", bufs=2) as bigpool,
            tc.tile_pool(name="pspool", bufs=8, space="PSUM") as pspool,
        ):
            w1D = wpool.tile([128, 2, 9, 64], FP8)
            w1L = wpool.tile([128, 9, 32], BF16)
            w2D = wpool.tile([96, 2, 3, 64], FP8)
            w2L = wpool.tile([96, 3, 2], FP8)
            onesL = wpool.tile([128, 3, 128], BF16)
            for dst, src in ((w1D, w1D_ext), (w1L, w1L_ext), (w2D, w2D_ext),
                             (w2L, w2L_ext), (onesL, onesL_ext)):
                nc.sync.dma_start(out=dst[:], in_=src[:])

            x_sb = [None] * PACKS
            arep = [None] * PACKS

            # x8 shares the 33.6KB/partition "bigpool" slots with the two arep
            # tiles: x8's last read (conv1 DR matmuls) ends before arep[1]'s
            # first write, so 2 bufs cover all three lifetimes.
            x8 = bigpool.tile([128, 2, BUF_W], FP8, tag="big", name="x8")
            nc.gpsimd.memset(x8[:, :, 0:FG], 0.0)
            nc.gpsimd.memset(x8[:, :, FG + S:BUF_W], 0.0)
            h64 = hpool.tile([64, BUF_W], FP8, name="h64")
            hrep4 = hpool.tile([96, 2, BUF_W], FP8, name="hrep4")
            for t in (h64, hrep4):
                nc.gpsimd.memset(t[:, ..., 0:FG], 0.0)
                nc.gpsimd.memset(t[:, ..., FG + S:BUF_W], 0.0)

            def alloc(p):
                x_sb[p] = xpool.tile([128, BUF_W], BF16, tag="xsb", name=f"xsb{p}")
                nc.gpsimd.memset(x_sb[p][:, 0:FG], 0.0)
                nc.gpsimd.memset(x_sb[p][:, FG + S:BUF_W], 0.0)

            def alloc_a(p):
                arep[p] = bigpool.tile([128, BUF_W], BF16, tag="big", name=f"arep{p}")
                # zero guards + the K-padding partitions (read by the box matmul)
                nc.gpsimd.memset(arep[p][:, :], 0.0)

            def load(p, c):
                # f32 -> bf16 cast DMA (SWDGE); chunks overlap by one row so
                # conv1_dr(c) depends only on chunk c of each pack
                r1 = min((c + 1) * CROWS + 1, H)
                nc.gpsimd.dma_start(
                    out=x_sb[p][:, _pos(c * CROWS, 0):_pos(r1, 0)],
                    in_=x_ext[2 * p:2 * p + 2, :, c * CROWS:r1]
                    .rearrange("b c h w -> (b c) (h w)"),
                )

            def cast8(p, c):
                # bf16 -> fp8 copy-cast, split across ScalarE/VectorE
                r1 = min((c + 1) * CROWS + 1, H)
                a, b = _pos(c * CROWS, 0), _pos(r1, 0)
                if (2 * p + c) % 2 == 0:
                    nc.scalar.activation(x8[:, p, a:b], x_sb[p][:, a:b],
                                         mybir.ActivationFunctionType.Copy)
                else:
                    nc.vector.tensor_copy(x8[:, p, a:b], x_sb[p][:, a:b])

            def conv1_dr(c):
                # tap-outer over groups of GRP tiles: weight loads amortize
                for g0 in range(c * TPC, (c + 1) * TPC, GRP):
                    phs = [pspool.tile([64, 512], F32, tag="ps", name="ph")
                           for _ in range(GRP)]
                    for t in range(9):
                        ky, kx = divmod(t, 3)
                        for j in range(GRP):
                            y0 = (g0 + j) * Y4
                            q = _pos(y0 + ky - 1, kx - 1)
                            nc.tensor.matmul(
                                phs[j][:, :],
                                w1D[:, :, t, :],
                                x8[:, :, q:q + 512],
                                perf_mode=mybir.MatmulPerfMode.DoubleRow,
                                start=(t == 0), stop=(t == 8),
                            )
                    for j in range(GRP):
                        y0 = (g0 + j) * Y4
                        # relu + cast, contiguous full-width rows (edge cols are
                        # wrap-garbage; overwritten by the edge eviction below)
                        nc.vector.tensor_scalar_max(
                            h64[:, _pos(y0, 0):_pos(y0, 0) + 512],
                            phs[j][:, :],
                            0.0,
                        )

            def conv1_edges(p, c):
                xs = x_sb[p]
                yc = c * CROWS
                for col, kxs in ((0, (1, 2)), (W - 1, (0, 1))):
                    pe = pspool.tile([32, CROWS], F32, tag="ps", name="pe")
                    first = True
                    for ky in range(3):
                        for kx in kxs:
                            nc.tensor.matmul(
                                pe[:, :].rearrange("p (y x) -> p y x", x=1),
                                w1L[:, ky * 3 + kx, :],
                                _view(xs, _pos(yc + ky - 1, col + kx - 1), CROWS, 1),
                                start=first, stop=(ky == 2 and kx == kxs[-1]),
                            )
                            first = False
                    nc.vector.tensor_scalar_max(
                        _view(h64, _pos(yc, col), CROWS, 1)[32 * p:32 * p + 32],
                        pe[:, :].rearrange("p (y x) -> p y x", x=1),
                        0.0,
                    )

            def repl_h(c):
                # Hrep4[(g,il,c), s][pos] = h_seg_s[pos + (g-1)*W]
                for s in range(2):
                    a0, b0 = _split(W, BUF_W)[c]
                    nc.sync.dma_start(out=hrep4[0:32, s, a0:b0],
                                      in_=h64[32 * s:32 * s + 32, a0 - W:b0 - W])
                    a1, b1 = _split(0, BUF_W)[c]
                    nc.sync.dma_start(out=hrep4[32:64, s, a1:b1],
                                      in_=h64[32 * s:32 * s + 32, a1:b1])
                    a2, b2 = _split(0, BUF_W - W)[c]
                    nc.sync.dma_start(out=hrep4[64:96, s, a2:b2],
                                      in_=h64[32 * s:32 * s + 32, a2 + W:b2 + W])

            def conv2(c):
                tiles = list(range(c * TPC, (c + 1) * TPC))
                pzs = [pspool.tile([64, 512], F32, tag="ps", name="pz")
                       for _ in tiles]
                for kx in range(3):
                    for j, rt in enumerate(tiles):
                        q = _pos(rt * Y4, kx - 1)
                        nc.tensor.matmul(
                            pzs[j][:, :],
                            w2D[:, :, kx, :],
                            hrep4[:, :, q:q + 512],
                            perf_mode=mybir.MatmulPerfMode.DoubleRow,
                            start=(kx == 0), stop=(kx == 2),
                        )
                for j, rt in enumerate(tiles):
                    y0 = rt * Y4
                    for p in range(PACKS):
                        # full-width sigmoid (edge cols garbage; fixed below)
                        nc.scalar.activation(
                            arep[p][0:2, _pos(y0, 0):_pos(y0, 0) + 512],
                            pzs[j][32 * p:32 * p + 2, :],
                            mybir.ActivationFunctionType.Sigmoid,
                        )

            def conv2_edges(p, c):
                yc = c * CROWS
                for col, kxs in ((0, (1, 2)), (W - 1, (0, 1))):
                    pz = pspool.tile([2, CROWS], F32, tag="ps", name="pze")
                    for j, kx in enumerate(kxs):
                        nc.tensor.matmul(
                            pz[:, :].rearrange("p (y x) -> p y x", x=1),
                            w2L[:, kx, :],
                            _view3(hrep4, p, _pos(yc, col + kx - 1), CROWS, 1),
                            start=(j == 0), stop=(j == len(kxs) - 1),
                        )
                    nc.scalar.activation(
                        _view(arep[p], _pos(yc, col), CROWS, 1)[0:2],
                        pz[:, :].rearrange("p (y x) -> p y x", x=1),
                        mybir.ActivationFunctionType.Sigmoid,
                    )

            def repl_a(p, c):
                ar = arep[p]
                for j in range(1, 9):
                    t = TAPORD[j]
                    o = (t // 3 - 1) * W + (t % 3 - 1)
                    if o > 0:
                        a, b = _split(0, BUF_W - o)[c]
                    else:
                        a, b = _split(-o, BUF_W)[c]
                    eng = nc.scalar if j % 2 else nc.sync
                    eng.dma_start(out=ar[2 * j:2 * j + 2, a:b],
                                  in_=ar[0:2, a + o:b + o])

            def box_mul(p, c):
                xs, ar = x_sb[p], arep[p]
                for rt in range(c * TPC, (c + 1) * TPC):
                    y0 = rt * Y4
                    pb = pspool.tile([128, 512], F32, tag="ps", name="pb")
                    nc.tensor.matmul(
                        pb[:, :],
                        onesL[:, 0, :],
                        ar[0:128, _pos(y0, 0):_pos(y0, 0) + 512],
                        start=True, stop=True,
                    )
                    v = _view(xs, _pos(y0, 1), Y4, XI)
                    nc.vector.tensor_mul(
                        v, v,
                        pb[:, :].rearrange("p (y x) -> p y x", y=Y4)[:, :, 1:127])
                yc = c * CROWS
                for e, col in ((1, 0), (2, W - 1)):
                    pb = pspool.tile([128, CROWS], F32, tag="ps", name="pbe")
                    nc.tensor.matmul(
                        pb[:, :].rearrange("p (y x) -> p y x", x=1),
                        onesL[:, e, :],
                        _view(ar, _pos(yc, col), CROWS, 1),
                        start=True, stop=True,
                    )
                    v = _view(xs, _pos(yc, col), CROWS, 1)
                    nc.vector.tensor_mul(
                        v, v, pb[:, :].rearrange("p (y x) -> p y x", x=1))

            def store(p, c):
                # bf16 -> f32 cast DMA (SWDGE), rows [c*16, (c+1)*16)
                nc.gpsimd.dma_start(
                    out=out_ext[2 * p:2 * p + 2, :, c * CROWS:(c + 1) * CROWS]
                    .rearrange("b c h w -> (b c) (h w)"),
                    in_=x_sb[p][:, _pos(c * CROWS, 0):_pos((c + 1) * CROWS, 0)],
                )

            # ---- emission (priority) order: chunk-interleaved across packs ----
            alloc(0)
            alloc(1)
            alloc_a(0)
            for c in range(NCH):
                load(0, c)
                cast8(0, c)
                load(1, c)
                cast8(1, c)

            conv1_dr(0)
            conv1_edges(0, 0)
            conv1_edges(1, 0)
            for c in range(1, NCH):
                conv1_dr(c)
                conv1_edges(0, c)
                conv1_edges(1, c)
                repl_h(c - 1)
            repl_h(NCH - 1)

            alloc_a(1)
            conv2(0)
            conv2_edges(0, 0)
            conv2_edges(1, 0)
            for c in range(1, NCH):
                conv2(c)
                conv2_edges(0, c)
                conv2_edges(1, c)
                repl_a(0, c - 1)
                repl_a(1, c - 1)
            repl_a(0, NCH - 1)
            repl_a(1, NCH - 1)

            for c in range(NCH):
                box_mul(0, c)
                store(0, c)
                box_mul(1, c)
                store(1, c)

    nc.compile()
    return nc


_CACHE = {}


def _get_nc():
    if "nc" not in _CACHE:
        _CACHE["nc"] = _build_nc()
    return _CACHE["nc"]


def _run(x, w1, w2, trace=False):
    x = np.ascontiguousarray(np.asarray(x, np.float32))
    w1D, w1L, w2D, w2L, onesL = _host_weights(w1, w2)
    nc = _get_nc()
    in_maps = []
    for k in range(N_CORES):
        in_maps.append({
            "x": x[k * B_LOC:(k + 1) * B_LOC],
            "w1D": w1D, "w1L": w1L, "w2D": w2D, "w2L": w2L, "onesL": onesL,
        })
    res = run_bass_kernel_spmd(nc, in_maps, core_ids=list(range(N_CORES)),
                               trace=trace)
    out = np.concatenate([r["out"] for r in res.results], axis=0)
    return out.astype(np.float32), res


def kernel(x, weights, w1, w2):
    out, _ = _run(x, w1, w2, trace=False)
    return out


def kernel_timed(x, weights, w1, w2):
    out, res = _run(x, w1, w2, trace=True)
    return out, res.exec_time_ns


# revision 17
# speedup vs baseline: 1.1119x; 1.1119x over previous
"""Trainium2 Bass kernel for ALayer: out = x * box3x3(sigmoid(conv3x3(relu(conv3x3(x,w1)),w2))).

Sharding: pure data parallel over batch (32 images -> 4 per core x 8 cores).

Per-core plan:
  - x loaded per 2-image pack as [128=(img2,c64), guarded spatial] bf16 (DMA cast),
    in 8 row-chunks; an fp8e4m3 copy x8 [128, 2, BUF_W] (both packs in the DoubleRow
    K-segment dim) is made by chunked ScalarE copy-casts (keeps DMA queues free).
  - conv1 interior: fp8 DoubleRow matmuls, K=256=(img4,c64), M=64=(pack2,img2,co16),
    9 taps accumulated in PSUM, tap-outer over groups of 4 tiles to amortize weight
    loads; full-width rows (contiguous rhs, wrap garbage at x=0/127) + bf16 per-pack
    edge-column matmuls with only the valid taps.
  - relu+cast eviction (DVE): ONE [64,512] op per tile into h64 [64=(pack,img,co), .]
    fp8; dy-replicas hrep4 [96=(dy3,img2,c16), 2seg, .] built by 6 chunked shifted
    DMAs; conv2 = 3 fp8-DR matmuls (dx), K=192, both packs at once.
  - sigmoid evictions (ACT) into group 0 of per-pack Arep [128-part, 18 live];
    8 shifted DMA replicas; box+channel-broadcast in ONE bf16 matmul: K=128
    (zero-padded), M=128=(img2,c64).
  - final multiply in-place on the x tile (DVE), stored with bf16->f32 DMA cast.

All stages emitted at 16-row chunk granularity so the Tile scheduler overlaps
DMA / PE / ACT / DVE across stage boundaries.
"""

import numpy as np
import ml_dtypes

import concourse.bacc as bacc
import concourse.mybir as mybir
from concourse.tile import TileContext
from concourse.bass_utils import run_bass_kernel_spmd

BF16 = mybir.dt.bfloat16
F32 = mybir.dt.float32
FP8 = mybir.dt.float8e4

# Problem constants (hardcoded; kernel.py must be self-contained)
B, C, H, W = 32, 64, 128, 128
N_CORES = 8
B_LOC = B // N_CORES            # 4 images per core
PACKS = B_LOC // 2              # 2 two-image packs
S = H * W                       # 16384
FG = 160                        # front zero-guard (>=129 needed)
BG = 288
BUF_W = FG + S + BG             # 16832 (16-aligned for DoubleRow AP steps)
Y4 = 4                          # output rows per psum tile
NT = H // Y4                    # 32 row tiles
GRP = 4                         # conv1 tiles per weight-reuse group
NCH = 8                         # row-chunks (16 rows each)
CROWS = H // NCH                # 16
TPC = NT // NCH                 # interior tiles per chunk (4)
XI = W - 2                      # 126 interior columns
TAPORD = (4, 0, 1, 2, 3, 5, 6, 7, 8)   # Arep group -> tap index (center first)


def _pos(y, x):
    return FG + y * W + x


def _host_weights(w1, w2):
    """Precompute SBUF weight layouts (block-diagonal over image packing)."""
    w1 = np.asarray(w1, np.float32)     # [16, 64, 3, 3]
    w2 = np.asarray(w2, np.float32)     # [1, 16, 3, 3]
    bf = ml_dtypes.bfloat16
    f8 = mybir.dt.np(FP8)

    # conv1 DoubleRow: lhsT[p=(il2,c64), seg2, t, m=(seg'2,il'2,co16)]
    #   = w1[co, c, ky, kx] * (seg'==seg) * (il'==il);  img = 2*seg + il
    w1D = np.zeros((128, 2, 9, 64), np.float32)
    for il in range(2):
        for seg in range(2):
            m0 = seg * 32 + il * 16
            for t in range(9):
                ky, kx = divmod(t, 3)
                w1D[il * 64:(il + 1) * 64, seg, t, m0:m0 + 16] = w1[:, :, ky, kx].T
    # conv1 edges (bf16, per pack): lhsT[p=(i2,c64), t, m=(il2,co16)]
    w1L = np.zeros((128, 9, 32), np.float32)
    for i in range(2):
        for t in range(9):
            ky, kx = divmod(t, 3)
            w1L[i * 64:(i + 1) * 64, t, i * 16:(i + 1) * 16] = w1[:, :, ky, kx].T
    # conv2 DoubleRow: lhsT[p=(g3,il2,c16), seg2, kx, m] = w2[0,c,g,kx] at
    # m = seg*32 + il (M=64 padded; psum rows {0,1,32,33} -> per-pack evictions)
    w2D = np.zeros((96, 2, 3, 64), np.float32)
    for g in range(3):
        for il in range(2):
            for seg in range(2):
                for kx in range(3):
                    w2D[g * 32 + il * 16:g * 32 + il * 16 + 16, seg, kx,
                        seg * 32 + il] = w2[0, :, g, kx]
    # conv2 edges (fp8 non-DR, per seg): lhsT[p=(g3,i2,c16), kx, il2]
    w2L = np.zeros((96, 3, 2), np.float32)
    for g in range(3):
        for i in range(2):
            for kx in range(3):
                w2L[g * 32 + i * 16:g * 32 + (i + 1) * 16, kx, i] = w2[0, :, g, kx]
    # box+bcast: lhsT[p=(group9,i2)+pad, e, m=(il2,c64)] = tap-allowed * (i == il)
    onesL = np.zeros((128, 3, 128), np.float32)
    for j, t in enumerate(TAPORD):
        kx = t % 3
        for i in range(2):
            p = 2 * j + i
            onesL[p, 0, i * 64:(i + 1) * 64] = 1.0
            if kx >= 1:
                onesL[p, 1, i * 64:(i + 1) * 64] = 1.0
            if kx <= 1:
                onesL[p, 2, i * 64:(i + 1) * 64] = 1.0
    return (w1D.astype(f8), w1L.astype(bf), w2D.astype(f8), w2L.astype(f8),
            onesL.astype(bf))


def _view(t, off, rows, cols):
    """AP over [partitions, (rows @ W-stride, cols @ 1)] at free offset `off`."""
    return t[:, off:off + (rows - 1) * W + 128].rearrange(
        "p (y x) -> p y x", x=W)[:, :, :cols]


def _view3(t, seg, off, rows, cols):
    """Same for a [P, 2, BUF_W] tensor at segment `seg`."""
    return t[:, seg, off:off + (rows - 1) * W + 128].rearrange(
        "p (y x) -> p y x", x=W)[:, :, :cols]


def _split(a, b, n=NCH):
    step = (b - a) // n
    cuts = [a + i * step for i in range(n)] + [b]
    return [(cuts[i], cuts[i + 1]) for i in range(n)]


def _build_nc():
    nc = bacc.Bacc(None, target_bir_lowering=False, debug=False)

    x_ext = nc.declare_dram_parameter("x", [B_LOC, C, H, W], F32, isOutput=False)
    out_ext = nc.declare_dram_parameter("out", [B_LOC, C, H, W], F32, isOutput=True)
    w1D_ext = nc.declare_dram_parameter("w1D", [128, 2, 9, 64], FP8, isOutput=False)
    w1L_ext = nc.declare_dram_parameter("w1L", [128, 9, 32], BF16, isOutput=False)
    w2D_ext = nc.declare_dram_parameter("w2D", [96, 2, 3, 64], FP8, isOutput=False)
    w2L_ext = nc.declare_dram_parameter("w2L", [96, 3, 2], FP8, isOutput=False)
    onesL_ext = nc.declare_dram_parameter("onesL", [128, 3, 128], BF16, isOutput=False)

    with TileContext(nc) as tc:
        with (
            tc.tile_pool(name="wpool", bufs=1) as wpool,
            tc.tile_pool(name="xpool", bufs=2) as xpool,
            tc.tile_pool(name="hpool", bufs=1) as hpool,
            tc.tile_pool(name="bigpool", bufs=2) as bigpool,
            tc.tile_pool(name="pspool", bufs=8, space="PSUM") as pspool,
        ):
            w1D = wpool.tile([128, 2, 9, 64], FP8)
            w1L = wpool.tile([128, 9, 32], BF16)
            w2D = wpool.tile([96, 2, 3, 64], FP8)
            w2L = wpool.tile([96, 3, 2], FP8)
            onesL = wpool.tile([128, 3, 128], BF16)
            for dst, src in ((w1D, w1D_ext), (w1L, w1L_ext), (w2D, w2D_ext),
                             (w2L, w2L_ext), (onesL, onesL_ext)):
                nc.sync.dma_start(out=dst[:], in_=src[:])

            x_sb = [None] * PACKS
            arep = [None] * PACKS

            # x8 shares the 33.6KB/partition "bigpool" slots with the two arep
            # tiles: x8's last read (conv1 DR matmuls) ends before arep[1]'s
            # first write, so 2 bufs cover all three lifetimes.
            x8 = bigpool.tile([128, 2, BUF_W], FP8, tag="big", name="x8")
            nc.vector.memset(x8[:, :, 0:FG], 0.0)
            nc.vector.memset(x8[:, :, FG + S:BUF_W], 0.0)
            h64 = hpool.tile([64, BUF_W], FP8, name="h64")
            hrep4 = hpool.tile([96, 2, BUF_W], FP8, name="hrep4")
            for t in (h64, hrep4):
                nc.vector.memset(t[:, ..., 0:FG], 0.0)
                nc.vector.memset(t[:, ..., FG + S:BUF_W], 0.0)

            def alloc(p):
                x_sb[p] = xpool.tile([128, BUF_W], BF16, tag="xsb", name=f"xsb{p}")
                nc.vector.memset(x_sb[p][:, 0:FG], 0.0)
                nc.vector.memset(x_sb[p][:, FG + S:BUF_W], 0.0)

            def alloc_a(p):
                arep[p] = bigpool.tile([128, BUF_W], BF16, tag="big", name=f"arep{p}")
                # zero guards + the K-padding partitions (read by the box matmul)
                nc.vector.memset(arep[p][:, :], 0.0)

            def load(p, c):
                # f32 -> bf16 cast DMA (SWDGE); chunks overlap by one row so
                # conv1_dr(c) depends only on chunk c of each pack
                r1 = min((c + 1) * CROWS + 1, H)
                nc.gpsimd.dma_start(
                    out=x_sb[p][:, _pos(c * CROWS, 0):_pos(r1, 0)],
                    in_=x_ext[2 * p:2 * p + 2, :, c * CROWS:r1]
                    .rearrange("b c h w -> (b c) (h w)"),
                )

            def cast8(p, c):
                # bf16 -> fp8 copy-cast, split across ScalarE/VectorE
                r1 = min((c + 1) * CROWS + 1, H)
                a, b = _pos(c * CROWS, 0), _pos(r1, 0)
                if (2 * p + c) % 2 == 0:
                    nc.scalar.activation(x8[:, p, a:b], x_sb[p][:, a:b],
                                         mybir.ActivationFunctionType.Copy)
                else:
                    nc.vector.tensor_copy(x8[:, p, a:b], x_sb[p][:, a:b])

            def conv1_dr(c):
                # tap-outer over groups of GRP tiles: weight loads amortize
                for g0 in range(c * TPC, (c + 1) * TPC, GRP):
                    phs = [pspool.tile([64, 512], F32, tag="ps", name="ph")
                           for _ in range(GRP)]
                    for t in range(9):
                        ky, kx = divmod(t, 3)
                        for j in range(GRP):
                            y0 = (g0 + j) * Y4
                            q = _pos(y0 + ky - 1, kx - 1)
                            nc.tensor.matmul(
                                phs[j][:, :],
                                w1D[:, :, t, :],
                                x8[:, :, q:q + 512],
                                perf_mode=mybir.MatmulPerfMode.DoubleRow,
                                start=(t == 0), stop=(t == 8),
                            )
                    for j in range(GRP):
                        y0 = (g0 + j) * Y4
                        # relu + cast, contiguous full-width rows (edge cols are
                        # wrap-garbage; overwritten by the edge eviction below)
                        nc.vector.tensor_scalar_max(
                            h64[:, _pos(y0, 0):_pos(y0, 0) + 512],
                            phs[j][:, :],
                            0.0,
                        )

            def conv1_edges(p, c):
                xs = x_sb[p]
                yc = c * CROWS
                for col, kxs in ((0, (1, 2)), (W - 1, (0, 1))):
                    pe = pspool.tile([32, CROWS], F32, tag="ps", name="pe")
                    first = True
                    for ky in range(3):
                        for kx in kxs:
                            nc.tensor.matmul(
                                pe[:, :].rearrange("p (y x) -> p y x", x=1),
                                w1L[:, ky * 3 + kx, :],
                                _view(xs, _pos(yc + ky - 1, col + kx - 1), CROWS, 1),
                                start=first, stop=(ky == 2 and kx == kxs[-1]),
                            )
                            first = False
                    nc.vector.tensor_scalar_max(
                        _view(h64, _pos(yc, col), CROWS, 1)[32 * p:32 * p + 32],
                        pe[:, :].rearrange("p (y x) -> p y x", x=1),
                        0.0,
                    )

            def repl_h(c):
                # Hrep4[(g,il,c), s][pos] = h_seg_s[pos + (g-1)*W]
                for s in range(2):
                    a0, b0 = _split(W, BUF_W)[c]
                    nc.sync.dma_start(out=hrep4[0:32, s, a0:b0],
                                      in_=h64[32 * s:32 * s + 32, a0 - W:b0 - W])
                    a1, b1 = _split(0, BUF_W)[c]
                    nc.sync.dma_start(out=hrep4[32:64, s, a1:b1],
                                      in_=h64[32 * s:32 * s + 32, a1:b1])
                    a2, b2 = _split(0, BUF_W - W)[c]
                    nc.sync.dma_start(out=hrep4[64:96, s, a2:b2],
                                      in_=h64[32 * s:32 * s + 32, a2 + W:b2 + W])

            def conv2(c):
                for rt in range(c * TPC, (c + 1) * TPC):
                    y0 = rt * Y4
                    pz = pspool.tile([64, 512], F32, tag="ps", name="pz")
                    for kx in range(3):
                        q = _pos(y0, kx - 1)
                        nc.tensor.matmul(
                            pz[:, :],
                            w2D[:, :, kx, :],
                            hrep4[:, :, q:q + 512],
                            perf_mode=mybir.MatmulPerfMode.DoubleRow,
                            start=(kx == 0), stop=(kx == 2),
                        )
                    for p in range(PACKS):
                        # full-width sigmoid (edge cols garbage; fixed below)
                        nc.scalar.activation(
                            arep[p][0:2, _pos(y0, 0):_pos(y0, 0) + 512],
                            pz[32 * p:32 * p + 2, :],
                            mybir.ActivationFunctionType.Sigmoid,
                        )

            def conv2_edges(p, c):
                yc = c * CROWS
                for col, kxs in ((0, (1, 2)), (W - 1, (0, 1))):
                    pz = pspool.tile([2, CROWS], F32, tag="ps", name="pze")
                    for j, kx in enumerate(kxs):
                        nc.tensor.matmul(
                            pz[:, :].rearrange("p (y x) -> p y x", x=1),
                            w2L[:, kx, :],
                            _view3(hrep4, p, _pos(yc, col + kx - 1), CROWS, 1),
                            start=(j == 0), stop=(j == len(kxs) - 1),
                        )
                    nc.scalar.activation(
                        _view(arep[p], _pos(yc, col), CROWS, 1)[0:2],
                        pz[:, :].rearrange("p (y x) -> p y x", x=1),
                        mybir.ActivationFunctionType.Sigmoid,
                    )

            def repl_a(p, c):
                ar = arep[p]
                for j in range(1, 9):
                    t = TAPORD[j]
                    o = (t // 3 - 1) * W + (t % 3 - 1)
                    if o > 0:
                        a, b = _split(0, BUF_W - o)[c]
                    else:
                        a, b = _split(-o, BUF_W)[c]
                    nc.sync.dma_start(out=ar[2 * j:2 * j + 2, a:b],
                                      in_=ar[0:2, a + o:b + o])

            def box_mul(p, c):
                xs, ar = x_sb[p], arep[p]
                for rt in range(c * TPC, (c + 1) * TPC):
                    y0 = rt * Y4
                    pb = pspool.tile([128, 504], F32, tag="ps", name="pb")
                    nc.tensor.matmul(
                        pb[:, :].rearrange("p (y x) -> p y x", y=Y4),
                        onesL[:, 0, :],
                        _view(ar, _pos(y0, 1), Y4, XI),
                        start=True, stop=True,
                    )
                    v = _view(xs, _pos(y0, 1), Y4, XI)
                    nc.vector.tensor_mul(
                        v, v, pb[:, :].rearrange("p (y x) -> p y x", y=Y4))
                yc = c * CROWS
                for e, col in ((1, 0), (2, W - 1)):
                    pb = pspool.tile([128, CROWS], F32, tag="ps", name="pbe")
                    nc.tensor.matmul(
                        pb[:, :].rearrange("p (y x) -> p y x", x=1),
                        onesL[:, e, :],
                        _view(ar, _pos(yc, col), CROWS, 1),
                        start=True, stop=True,
                    )
                    v = _view(xs, _pos(yc, col), CROWS, 1)
                    nc.vector.tensor_mul(
                        v, v, pb[:, :].rearrange("p (y x) -> p y x", x=1))

            def store(p, c):
                # bf16 -> f32 cast DMA (SWDGE), rows [c*16, (c+1)*16)
                nc.gpsimd.dma_start(
                    out=out_ext[2 * p:2 * p + 2, :, c * CROWS:(c + 1) * CROWS]
                    .rearrange("b c h w -> (b c) (h w)"),
                    in_=x_sb[p][:, _pos(c * CROWS, 0):_pos((c + 1) * CROWS, 0)],
                )

            # ---- emission (priority) order: chunk-interleaved across packs ----
            alloc(0)
            alloc(1)
            alloc_a(0)
            for c in range(NCH):
                load(0, c)
                cast8(0, c)
                load(1, c)
                cast8(1, c)

            conv1_dr(0)
            conv1_edges(0, 0)
            conv1_edges(1, 0)
            for c in range(1, NCH):
                conv1_dr(c)
                conv1_edges(0, c)
                conv1_edges(1, c)
                repl_h(c - 1)
            repl_h(NCH - 1)

            alloc_a(1)
            conv2(0)
            conv2_edges(0, 0)
            conv2_edges(1, 0)
            for c in range(1, NCH):
                conv2(c)
                conv2_edges(0, c)
                conv2_edges(1, c)
                repl_a(0, c - 1)
                repl_a(1, c - 1)
            repl_a(0, NCH - 1)
            repl_a(1, NCH - 1)

            for c in range(NCH):
                box_mul(0, c)
                store(0, c)
                box_mul(1, c)
                store(1, c)

    nc.compile()
    return nc


_CACHE = {}


def _get_nc():
    if "nc" not in _CACHE:
        _CACHE["nc"] = _build_nc()
    return _CACHE["nc"]


def _run(x, w1, w2, trace=False):
    x = np.ascontiguousarray(np.asarray(x, np.float32))
    w1D, w1L, w2D, w2L, onesL = _host_weights(w1, w2)
    nc = _get_nc()
    in_maps = []
    for k in range(N_CORES):
        in_maps.append({
            "x": x[k * B_LOC:(k + 1) * B_LOC],
            "w1D": w1D, "w1L": w1L, "w2D": w2D, "w2L": w2L, "onesL": onesL,
        })
    res = run_bass_kernel_spmd(nc, in_maps, core_ids=list(range(N_CORES)),
                               trace=trace)
    out = np.concatenate([r["out"] for r in res.results], axis=0)
    return out.astype(np.float32), res


def kernel(x, weights, w1, w2):
    out, _ = _run(x, w1, w2, trace=False)
    return out


def kernel_timed(x, weights, w1, w2):
    out, res = _run(x, w1, w2, trace=True)
    return out, res.exec_time_ns


# revision 18
# speedup vs baseline: 1.1486x; 1.0330x over previous
"""Trainium2 Bass kernel for ALayer: out = x * box3x3(sigmoid(conv3x3(relu(conv3x3(x,w1)),w2))).

Sharding: pure data parallel over batch (32 images -> 4 per core x 8 cores).

Per-core plan:
  - x loaded per 2-image pack as [128=(img2,c64), guarded spatial] bf16 (DMA cast),
    in 8 row-chunks; an fp8e4m3 copy x8 [128, 2, BUF_W] (both packs in the DoubleRow
    K-segment dim) is made by chunked ScalarE copy-casts (keeps DMA queues free).
  - conv1 interior: fp8 DoubleRow matmuls, K=256=(img4,c64), M=64=(pack2,img2,co16),
    9 taps accumulated in PSUM, tap-outer over groups of 4 tiles to amortize weight
    loads; full-width rows (contiguous rhs, wrap garbage at x=0/127) + bf16 per-pack
    edge-column matmuls with only the valid taps.
  - relu+cast eviction (DVE): ONE [64,512] op per tile into h64 [64=(pack,img,co), .]
    fp8; dy-replicas hrep4 [96=(dy3,img2,c16), 2seg, .] built by 6 chunked shifted
    DMAs; conv2 = 3 fp8-DR matmuls (dx), K=192, both packs at once.
  - sigmoid evictions (ACT) into group 0 of per-pack Arep [128-part, 18 live];
    8 shifted DMA replicas; box+channel-broadcast in ONE bf16 matmul: K=128
    (zero-padded), M=128=(img2,c64).
  - final multiply in-place on the x tile (DVE), stored with bf16->f32 DMA cast.

All stages emitted at 16-row chunk granularity so the Tile scheduler overlaps
DMA / PE / ACT / DVE across stage boundaries.
"""

import numpy as np
import ml_dtypes

import concourse.bacc as bacc
import concourse.mybir as mybir
from concourse.tile import TileContext
from concourse.bass_utils import run_bass_kernel_spmd

BF16 = mybir.dt.bfloat16
F32 = mybir.dt.float32
FP8 = mybir.dt.float8e4

# Problem constants (hardcoded; kernel.py must be self-contained)
B, C, H, W = 32, 64, 128, 128
N_CORES = 8
B_LOC = B // N_CORES            # 4 images per core
PACKS = B_LOC // 2              # 2 two-image packs
S = H * W                       # 16384
FG = 160                        # front zero-guard (>=129 needed)
BG = 288
BUF_W = FG + S + BG             # 16832 (16-aligned for DoubleRow AP steps)
Y4 = 4                          # output rows per psum tile
NT = H // Y4                    # 32 row tiles
GRP = 4                         # conv1 tiles per weight-reuse group
NCH = 8                         # row-chunks (16 rows each)
CROWS = H // NCH                # 16
TPC = NT // NCH                 # interior tiles per chunk (4)
XI = W - 2                      # 126 interior columns
TAPORD = (4, 0, 1, 2, 3, 5, 6, 7, 8)   # Arep group -> tap index (center first)


def _pos(y, x):
    return FG + y * W + x


def _host_weights(w1, w2):
    """Precompute SBUF weight layouts (block-diagonal over image packing)."""
    w1 = np.asarray(w1, np.float32)     # [16, 64, 3, 3]
    w2 = np.asarray(w2, np.float32)     # [1, 16, 3, 3]
    bf = ml_dtypes.bfloat16
    f8 = mybir.dt.np(FP8)

    # conv1 DoubleRow: lhsT[p=(il2,c64), seg2, t, m=(seg'2,il'2,co16)]
    #   = w1[co, c, ky, kx] * (seg'==seg) * (il'==il);  img = 2*seg + il
    w1D = np.zeros((128, 2, 9, 64), np.float32)
    for il in range(2):
        for seg in range(2):
            m0 = seg * 32 + il * 16
            for t in range(9):
                ky, kx = divmod(t, 3)
                w1D[il * 64:(il + 1) * 64, seg, t, m0:m0 + 16] = w1[:, :, ky, kx].T
    # conv1 edges (bf16, per pack): lhsT[p=(i2,c64), t, m=(il2,co16)]
    w1L = np.zeros((128, 9, 32), np.float32)
    for i in range(2):
        for t in range(9):
            ky, kx = divmod(t, 3)
            w1L[i * 64:(i + 1) * 64, t, i * 16:(i + 1) * 16] = w1[:, :, ky, kx].T
    # conv2 DoubleRow: lhsT[p=(g3,il2,c16), seg2, kx, m] = w2[0,c,g,kx] at
    # m = seg*32 + il (M=64 padded; psum rows {0,1,32,33} -> per-pack evictions)
    w2D = np.zeros((96, 2, 3, 64), np.float32)
    for g in range(3):
        for il in range(2):
            for seg in range(2):
                for kx in range(3):
                    w2D[g * 32 + il * 16:g * 32 + il * 16 + 16, seg, kx,
                        seg * 32 + il] = w2[0, :, g, kx]
    # conv2 edges (fp8 non-DR, per seg): lhsT[p=(g3,i2,c16), kx, il2]
    w2L = np.zeros((96, 3, 2), np.float32)
    for g in range(3):
        for i in range(2):
            for kx in range(3):
                w2L[g * 32 + i * 16:g * 32 + (i + 1) * 16, kx, i] = w2[0, :, g, kx]
    # box+bcast: lhsT[p=32*pk+(group9,i2)+pad, e, m=(il2,c64)] = tap-allowed*(i==il)
    # two identical 32-row blocks so lhsT/rhs partition bases match per pack
    onesL = np.zeros((64, 3, 128), np.float32)
    for pk in range(2):
        for j, t in enumerate(TAPORD):
            kx = t % 3
            for i in range(2):
                p = 32 * pk + 2 * j + i
                onesL[p, 0, i * 64:(i + 1) * 64] = 1.0
                if kx >= 1:
                    onesL[p, 1, i * 64:(i + 1) * 64] = 1.0
                if kx <= 1:
                    onesL[p, 2, i * 64:(i + 1) * 64] = 1.0
    return (w1D.astype(f8), w1L.astype(bf), w2D.astype(f8), w2L.astype(f8),
            onesL.astype(bf))


def _view(t, off, rows, cols):
    """AP over [partitions, (rows @ W-stride, cols @ 1)] at free offset `off`."""
    return t[:, off:off + (rows - 1) * W + 128].rearrange(
        "p (y x) -> p y x", x=W)[:, :, :cols]


def _view3(t, seg, off, rows, cols):
    """Same for a [P, 2, BUF_W] tensor at segment `seg`."""
    return t[:, seg, off:off + (rows - 1) * W + 128].rearrange(
        "p (y x) -> p y x", x=W)[:, :, :cols]


def _split(a, b, n=NCH):
    step = (b - a) // n
    cuts = [a + i * step for i in range(n)] + [b]
    return [(cuts[i], cuts[i + 1]) for i in range(n)]


def _build_nc():
    nc = bacc.Bacc(None, target_bir_lowering=False, debug=False)

    x_ext = nc.declare_dram_parameter("x", [B_LOC, C, H, W], F32, isOutput=False)
    out_ext = nc.declare_dram_parameter("out", [B_LOC, C, H, W], F32, isOutput=True)
    w1D_ext = nc.declare_dram_parameter("w1D", [128, 2, 9, 64], FP8, isOutput=False)
    w1L_ext = nc.declare_dram_parameter("w1L", [128, 9, 32], BF16, isOutput=False)
    w2D_ext = nc.declare_dram_parameter("w2D", [96, 2, 3, 64], FP8, isOutput=False)
    w2L_ext = nc.declare_dram_parameter("w2L", [96, 3, 2], FP8, isOutput=False)
    onesL_ext = nc.declare_dram_parameter("onesL", [64, 3, 128], BF16, isOutput=False)

    with TileContext(nc) as tc:
        with (
            tc.tile_pool(name="wpool", bufs=1) as wpool,
            tc.tile_pool(name="xpool", bufs=2) as xpool,
            tc.tile_pool(name="hpool", bufs=1) as hpool,
            tc.tile_pool(name="bigpool", bufs=2) as bigpool,
            tc.tile_pool(name="pspool", bufs=8, space="PSUM") as pspool,
        ):
            w1D = wpool.tile([128, 2, 9, 64], FP8)
            w1L = wpool.tile([128, 9, 32], BF16)
            w2D = wpool.tile([96, 2, 3, 64], FP8)
            w2L = wpool.tile([96, 3, 2], FP8)
            onesL = wpool.tile([64, 3, 128], BF16)
            for dst, src in ((w1D, w1D_ext), (w1L, w1L_ext), (w2D, w2D_ext),
                             (w2L, w2L_ext), (onesL, onesL_ext)):
                nc.sync.dma_start(out=dst[:], in_=src[:])

            x_sb = [None] * PACKS

            x8 = bigpool.tile([128, 2, BUF_W], FP8, tag="big", name="x8")
            nc.vector.memset(x8[:, :, 0:FG], 0.0)
            nc.vector.memset(x8[:, :, FG + S:BUF_W], 0.0)
            # both packs' attention maps in one tensor: pack p = rows 32p+
            # (group j at rows 32p+2j, 32p+2j+1; rows 32p+18..32p+31 stay
            # zero as the box matmul's K=32 padding)
            A2 = bigpool.tile([64, BUF_W], BF16, tag="big", name="A2")
            nc.vector.memset(A2[:, :], 0.0)
            h64 = hpool.tile([64, BUF_W], FP8, name="h64")
            hrep4 = hpool.tile([96, 2, BUF_W], FP8, name="hrep4")
            for t in (h64, hrep4):
                nc.vector.memset(t[:, ..., 0:FG], 0.0)
                nc.vector.memset(t[:, ..., FG + S:BUF_W], 0.0)

            def alloc(p):
                x_sb[p] = xpool.tile([128, BUF_W], BF16, tag="xsb", name=f"xsb{p}")
                nc.vector.memset(x_sb[p][:, 0:FG], 0.0)
                nc.vector.memset(x_sb[p][:, FG + S:BUF_W], 0.0)

            def load(p, c):
                # f32 -> bf16 cast DMA (SWDGE); chunks overlap by one row so
                # conv1_dr(c) depends only on chunk c of each pack
                r1 = min((c + 1) * CROWS + 1, H)
                nc.gpsimd.dma_start(
                    out=x_sb[p][:, _pos(c * CROWS, 0):_pos(r1, 0)],
                    in_=x_ext[2 * p:2 * p + 2, :, c * CROWS:r1]
                    .rearrange("b c h w -> (b c) (h w)"),
                )

            def cast8(p, c):
                # bf16 -> fp8 copy-cast, split across ScalarE/VectorE
                r1 = min((c + 1) * CROWS + 1, H)
                a, b = _pos(c * CROWS, 0), _pos(r1, 0)
                if (2 * p + c) % 2 == 0:
                    nc.scalar.activation(x8[:, p, a:b], x_sb[p][:, a:b],
                                         mybir.ActivationFunctionType.Copy)
                else:
                    nc.vector.tensor_copy(x8[:, p, a:b], x_sb[p][:, a:b])

            def conv1_dr(c):
                # tap-outer over groups of GRP tiles: weight loads amortize
                for g0 in range(c * TPC, (c + 1) * TPC, GRP):
                    phs = [pspool.tile([64, 512], F32, tag="ps", name="ph")
                           for _ in range(GRP)]
                    for t in range(9):
                        ky, kx = divmod(t, 3)
                        for j in range(GRP):
                            y0 = (g0 + j) * Y4
                            q = _pos(y0 + ky - 1, kx - 1)
                            nc.tensor.matmul(
                                phs[j][:, :],
                                w1D[:, :, t, :],
                                x8[:, :, q:q + 512],
                                perf_mode=mybir.MatmulPerfMode.DoubleRow,
                                start=(t == 0), stop=(t == 8),
                            )
                    for j in range(GRP):
                        y0 = (g0 + j) * Y4
                        # relu + cast, contiguous full-width rows (edge cols are
                        # wrap-garbage; overwritten by the edge eviction below)
                        nc.vector.tensor_scalar_max(
                            h64[:, _pos(y0, 0):_pos(y0, 0) + 512],
                            phs[j][:, :],
                            0.0,
                        )

            def conv1_edges(p, c):
                xs = x_sb[p]
                yc = c * CROWS
                for col, kxs in ((0, (1, 2)), (W - 1, (0, 1))):
                    pe = pspool.tile([32, CROWS], F32, tag="ps", name="pe")
                    first = True
                    for ky in range(3):
                        for kx in kxs:
                            nc.tensor.matmul(
                                pe[:, :].rearrange("p (y x) -> p y x", x=1),
                                w1L[:, ky * 3 + kx, :],
                                _view(xs, _pos(yc + ky - 1, col + kx - 1), CROWS, 1),
                                start=first, stop=(ky == 2 and kx == kxs[-1]),
                            )
                            first = False
                    nc.vector.tensor_scalar_max(
                        _view(h64, _pos(yc, col), CROWS, 1)[32 * p:32 * p + 32],
                        pe[:, :].rearrange("p (y x) -> p y x", x=1),
                        0.0,
                    )

            def repl_h(c):
                # Hrep4[(g,il,c), s][pos] = h_seg_s[pos + (g-1)*W]
                for s in range(2):
                    a0, b0 = _split(W, BUF_W)[c]
                    nc.sync.dma_start(out=hrep4[0:32, s, a0:b0],
                                      in_=h64[32 * s:32 * s + 32, a0 - W:b0 - W])
                    a1, b1 = _split(0, BUF_W)[c]
                    nc.sync.dma_start(out=hrep4[32:64, s, a1:b1],
                                      in_=h64[32 * s:32 * s + 32, a1:b1])
                    a2, b2 = _split(0, BUF_W - W)[c]
                    nc.sync.dma_start(out=hrep4[64:96, s, a2:b2],
                                      in_=h64[32 * s:32 * s + 32, a2 + W:b2 + W])

            def conv2(c):
                for rt in range(c * TPC, (c + 1) * TPC):
                    y0 = rt * Y4
                    pz = pspool.tile([64, 512], F32, tag="ps", name="pz")
                    for kx in range(3):
                        q = _pos(y0, kx - 1)
                        nc.tensor.matmul(
                            pz[:, :],
                            w2D[:, :, kx, :],
                            hrep4[:, :, q:q + 512],
                            perf_mode=mybir.MatmulPerfMode.DoubleRow,
                            start=(kx == 0), stop=(kx == 2),
                        )
                    for p in range(PACKS):
                        # full-width sigmoid (edge cols garbage; fixed below)
                        nc.scalar.activation(
                            A2[32 * p:32 * p + 2, _pos(y0, 0):_pos(y0, 0) + 512],
                            pz[32 * p:32 * p + 2, :],
                            mybir.ActivationFunctionType.Sigmoid,
                        )

            def conv2_edges(p, c):
                yc = c * CROWS
                for col, kxs in ((0, (1, 2)), (W - 1, (0, 1))):
                    pz = pspool.tile([2, CROWS], F32, tag="ps", name="pze")
                    for j, kx in enumerate(kxs):
                        nc.tensor.matmul(
                            pz[:, :].rearrange("p (y x) -> p y x", x=1),
                            w2L[:, kx, :],
                            _view3(hrep4, p, _pos(yc, col + kx - 1), CROWS, 1),
                            start=(j == 0), stop=(j == len(kxs) - 1),
                        )
                    nc.scalar.activation(
                        _view(A2, _pos(yc, col), CROWS, 1)[32 * p:32 * p + 2],
                        pz[:, :].rearrange("p (y x) -> p y x", x=1),
                        mybir.ActivationFunctionType.Sigmoid,
                    )

            def repl_a(p, c):
                q = 32 * p
                for j in range(1, 9):
                    t = TAPORD[j]
                    o = (t // 3 - 1) * W + (t % 3 - 1)
                    if o > 0:
                        a, b = _split(0, BUF_W - o)[c]
                    else:
                        a, b = _split(-o, BUF_W)[c]
                    nc.sync.dma_start(out=A2[q + 2 * j:q + 2 * j + 2, a:b],
                                      in_=A2[q:q + 2, a + o:b + o])

            def box_mul(p, c):
                xs, q = x_sb[p], 32 * p
                for rt in range(c * TPC, (c + 1) * TPC):
                    y0 = rt * Y4
                    pb = pspool.tile([128, 504], F32, tag="ps", name="pb")
                    nc.tensor.matmul(
                        pb[:, :].rearrange("p (y x) -> p y x", y=Y4),
                        onesL[q:q + 32, 0, :],
                        _view(A2, _pos(y0, 1), Y4, XI)[q:q + 32],
                        start=True, stop=True,
                    )
                    v = _view(xs, _pos(y0, 1), Y4, XI)
                    nc.vector.tensor_mul(
                        v, v, pb[:, :].rearrange("p (y x) -> p y x", y=Y4))
                yc = c * CROWS
                for e, col in ((1, 0), (2, W - 1)):
                    pb = pspool.tile([128, CROWS], F32, tag="ps", name="pbe")
                    nc.tensor.matmul(
                        pb[:, :].rearrange("p (y x) -> p y x", x=1),
                        onesL[q:q + 32, e, :],
                        _view(A2, _pos(yc, col), CROWS, 1)[q:q + 32],
                        start=True, stop=True,
                    )
                    v = _view(xs, _pos(yc, col), CROWS, 1)
                    nc.vector.tensor_mul(
                        v, v, pb[:, :].rearrange("p (y x) -> p y x", x=1))

            def store(p, c):
                # bf16 -> f32 cast DMA (SWDGE), rows [c*16, (c+1)*16)
                nc.gpsimd.dma_start(
                    out=out_ext[2 * p:2 * p + 2, :, c * CROWS:(c + 1) * CROWS]
                    .rearrange("b c h w -> (b c) (h w)"),
                    in_=x_sb[p][:, _pos(c * CROWS, 0):_pos((c + 1) * CROWS, 0)],
                )

            # ---- emission (priority) order: chunk-interleaved across packs ----
            alloc(0)
            alloc(1)
            for c in range(NCH):
                load(0, c)
                cast8(0, c)
                load(1, c)
                cast8(1, c)

            conv1_dr(0)
            conv1_edges(0, 0)
            conv1_edges(1, 0)
            for c in range(1, NCH):
                conv1_dr(c)
                conv1_edges(0, c)
                conv1_edges(1, c)
                repl_h(c - 1)
            repl_h(NCH - 1)

            conv2(0)
            conv2_edges(0, 0)
            conv2_edges(1, 0)
            for c in range(1, NCH):
                conv2(c)
                conv2_edges(0, c)
                conv2_edges(1, c)
                repl_a(0, c - 1)
                repl_a(1, c - 1)
            repl_a(0, NCH - 1)
            repl_a(1, NCH - 1)

            for c in range(NCH):
                box_mul(0, c)
                store(0, c)
                box_mul(1, c)
                store(1, c)

    nc.compile()
    return nc


_CACHE = {}


def _get_nc():
    if "nc" not in _CACHE:
        _CACHE["nc"] = _build_nc()
    return _CACHE["nc"]


def _run(x, w1, w2, trace=False):
    x = np.ascontiguousarray(np.asarray(x, np.float32))
    w1D, w1L, w2D, w2L, onesL = _host_weights(w1, w2)
    nc = _get_nc()
    in_maps = []
    for k in range(N_CORES):
        in_maps.append({
            "x": x[k * B_LOC:(k + 1) * B_LOC],
            "w1D": w1D, "w1L": w1L, "w2D": w2D, "w2L": w2L, "onesL": onesL,
        })
    res = run_bass_kernel_spmd(nc, in_maps, core_ids=list(range(N_CORES)),
                               trace=trace)
    out = np.concatenate([r["out"] for r in res.results], axis=0)
    return out.astype(np.float32), res


def kernel(x, weights, w1, w2):
    out, _ = _run(x, w1, w2, trace=False)
    return out


def kernel_timed(x, weights, w1, w2):
    out, res = _run(x, w1, w2, trace=True)
    return out, res.exec_time_ns
